# revision 15
# baseline (speedup 1.0000x reference)
"""Trainium2 Bass kernel for DeformationNetworkGraphConvolutionalFullRes.

Full (unsharded) inputs in, full output out. Data-parallel over the 4 meshes:
core m processes mesh m (cores 4-7 idle). Inside each core:

  - vert_align sampling as (S @ F) @ W == S @ (F @ W): per feature map,
    F[C,HW] @ Wslice[C,128] -> G[HW,128] (bf16 matmuls), then the sparse
    bilinear operator S applied as dense [128px, 512vert] bf16 blocks
    (built host-side) accumulated in PSUM. Vertices pre-sorted by image cell.
  - Each GraphConv layer splits message traffic across two lanes:
      * DMA lane (dst tiles 0..R3T0-1): h1 rows -> HBM (partition-major
        contiguous layout), dma_gather pulls messages in dst-sorted order,
        segmented sum via one-hot (is_equal) matmuls accumulating in PSUM
        on top of h0, ReLU writes next activations.
      * Pool lane (last K_R3 dst tiles): h1 kept in column form [128,VP]
        f32 in SBUF; gpsimd.ap_gather pulls message columns; bf16 msg pairs
        scatter-add (d=2, unique dst per call via rank-pass decomposition)
        into a 2-lane accumulator initialized with h0; lanes summed + ReLU.
"""

import ml_dtypes
import numpy as np
from contextlib import ExitStack

import concourse.bass as bass
import concourse.tile as tile
from concourse import bacc, mybir
from concourse.bass_utils import run_bass_kernel_spmd

# ---------------- problem constants (hardcoded per spec) ----------------
B = 4
V = 10242
E_PER = 30720
HID = 128
MAPS = [(256, 56), (512, 28), (1024, 14), (2048, 7)]  # (C, H==W)
CH_OFF = [0, 256, 768, 1792, 3840]

VP = 10752            # padded vertex count: 84 tiles of 128
NT = VP // 128        # 84 vertex tiles
NVCH = VP // 512      # 21 vertex chunks (sampling)
NSUB = 7              # max edge subchunks (of 128) per dst tile
K_R3 = 0              # dst tiles handled by the Pool lane (last K_R3 tiles)
R3T0 = NT - K_R3      # first pool-lane tile
R3V = K_R3 * 128      # pool-lane vertex slots
GT = 4                # dst tiles per dma-gather group
NGRP = R3T0 // GT     # gather groups (DMA lane)
SUB_G = GT * NSUB
HB = 12               # h1 row write batch (tiles)
CH_R3 = 1024          # pool-lane msg chunk (slots, = 2*pairs)
ZCOL = VP - 2         # guaranteed-zero h1col column (global sorted-slot)
DUMT = VP - 1         # dummy dst (global sorted-slot; pad region)

F32 = mybir.dt.float32
BF16 = mybir.dt.bfloat16
FP16 = mybir.dt.float16
I32 = mybir.dt.int32
I16 = mybir.dt.int16
AF = mybir.ActivationFunctionType


def _corners(grid, W):
    x = (grid[:, 0] + 1.0) * 0.5 * (W - 1)
    y = (grid[:, 1] + 1.0) * 0.5 * (W - 1)
    x0f, y0f = np.floor(x), np.floor(y)
    wx1, wy1 = (x - x0f).astype(np.float32), (y - y0f).astype(np.float32)
    wx0, wy0 = 1.0 - wx1, 1.0 - wy1
    x0 = np.clip(x0f, 0, W - 1).astype(np.int64)
    x1 = np.clip(x0f + 1, 0, W - 1).astype(np.int64)
    y0 = np.clip(y0f, 0, W - 1).astype(np.int64)
    y1 = np.clip(y0f + 1, 0, W - 1).astype(np.int64)
    return [
        (y0 * W + x0, wy0 * wx0),
        (y0 * W + x1, wy0 * wx1),
        (y1 * W + x0, wy1 * wx0),
        (y1 * W + x1, wy1 * wx1),
    ]


def _wrap16(a):
    """[n] int -> [128, n//16] int16 (wrapped in 16 partitions, replicated)."""
    return np.tile(np.asarray(a, np.int16).reshape(-1, 16).T, (8, 1))


def _prep(inputs):
    feats = [inputs["feat1"], inputs["feat2"], inputs["feat3"], inputs["feat4"]]
    av = np.asarray(inputs["aligned_verts"], np.float32)
    verts = np.asarray(inputs["verts_packed"], np.float32)
    enc = np.asarray(inputs["image_enc"], np.float32)
    edges = np.asarray(inputs["edges"], np.int64)

    for bn in ["bottleneck_b", "g0_b0", "g0_b1", "off_b"]:
        assert not np.any(np.asarray(inputs[bn])), f"{bn} nonzero: unsupported"
    assert not np.any(np.asarray(inputs["gb0"])) and not np.any(
        np.asarray(inputs["gb1"])
    ), "gb nonzero: unsupported"

    # per-mesh vertex sort (by finest-map cell) ----------------------------
    sigmas, invs, corners_all = [], [], []
    for m in range(B):
        grid = av[m, :, :2]
        cs = _corners(grid, MAPS[0][1])
        key = cs[0][0]
        sigma = np.argsort(key, kind="stable")
        inv = np.empty(V, np.int64)
        inv[sigma] = np.arange(V)
        sigmas.append(sigma)
        invs.append(inv)
        corners_all.append(
            [[(pix[sigma], w[sigma]) for (pix, w) in _corners(grid, Wm)]
             for (_, Wm) in MAPS]
        )

    # sampling schedule ----------------------------------------------------
    ntile_map = [(Wm * Wm + 127) // 128 for (_, Wm) in MAPS]
    g_off = np.cumsum([0] + ntile_map)
    sched = []
    for mi in range(4):
        per_c = []
        for c in range(NVCH):
            lo, hi = c * 512, min((c + 1) * 512, V)
            tiles = set()
            if lo < V:
                for m in range(B):
                    for (pix, _w) in corners_all[m][mi]:
                        pc = pix[lo:hi] // 128
                        tiles.update(np.unique(pc).tolist())
            per_c.append(sorted(tiles) if tiles else [0])
        np_m = max(len(t) for t in per_c)
        per_c = [t + [t[0]] * (np_m - len(t)) for t in per_c]
        sched.append(per_c)
    np_list = [len(sched[mi][0]) for mi in range(4)]
    npair = sum(np_list) * NVCH

    # graph structure ------------------------------------------------------
    esorted = []   # per mesh: (dst, src) directed, dst-sorted (r1 part only)
    ecnts = []     # per mesh: [NT] counts (r1 tiles only meaningful)
    r3_lists = []  # per mesh: {dst_local: [srcs]} for pool-lane tiles
    for m in range(B):
        e = edges[m * E_PER:(m + 1) * E_PER] - m * V
        a = invs[m][e[:, 0]]
        b = invs[m][e[:, 1]]
        dst = np.concatenate([a, b])
        src = np.concatenate([b, a])
        r3_mask = dst >= R3T0 * 128
        d1, s1 = dst[~r3_mask], src[~r3_mask]
        order = np.lexsort((s1, d1))
        esorted.append((d1[order], s1[order]))
        ecnts.append(np.bincount(d1 // 128, minlength=NT))
        d3 = dst[r3_mask] - R3T0 * 128
        s3 = src[r3_mask]
        per_dst = {}
        o3 = np.lexsort((s3, d3))
        for dd, ss in zip(d3[o3], s3[o3]):
            per_dst.setdefault(int(dd), []).append(int(ss))
        r3_lists.append(per_dst)

    # per-group slot packing (GT tiles per group, contiguous dst-sorted)
    gcnts = [ecnts[m][:R3T0].reshape(NGRP, GT).sum(axis=1) for m in range(B)]
    ng_g = np.maximum(1, -(-np.stack(gcnts).max(axis=0) // 128))  # [NGRP]
    goff = np.concatenate([[0], np.cumsum(ng_g)]).astype(int)
    tot_sub = int(goff[-1])
    # per (group, tile) subchunk spans (union over meshes)
    spans = []
    for g in range(NGRP):
        per_tile = []
        for ti in range(GT):
            lo = hi = None
            for m in range(B):
                ct = ecnts[m][:R3T0]
                start = int(ct[g * GT:g * GT + ti].sum())
                end = start + int(ct[g * GT + ti])
                if end > start:
                    l0, h0 = start // 128, -(-end // 128)
                    lo = l0 if lo is None else min(lo, l0)
                    hi = h0 if hi is None else max(hi, h0)
            per_tile.append((lo, hi) if lo is not None else None)
        spans.append(per_tile)
    span_tot = [sum(h - l for sp in spans[g] if sp for (l, h) in [sp])
                for g in range(NGRP)]
    # resident one-hot budget: groups 0..G_RES-1 prebuilt in SBUF
    RES_MAX = 312
    G_RES = 0
    acc = 0
    for g in range(NGRP):
        if acc + span_tot[g] > RES_MAX:
            break
        acc += span_tot[g]
        G_RES = g + 1
    res_slots = sum(span_tot[:G_RES])

    # pool-lane pass/chunk schedule (SPMD-uniform sizes) -------------------
    maxdeg = 0
    for m in range(B):
        for v in r3_lists[m].values():
            maxdeg = max(maxdeg, len(v))
    npass = (maxdeg + 1) // 2
    pass_pairs = []  # per pass: max over meshes of #dsts with deg > 2k
    for k in range(npass):
        mx = 0
        for m in range(B):
            mx = max(mx, sum(1 for v in r3_lists[m].values() if len(v) > 2 * k))
        pass_pairs.append(mx)
    # chunks: (pass, pair_start, npairs) with one dummy pair at slot 0 each
    chunks = []
    for k in range(npass):
        rem = pass_pairs[k]
        pos = 0
        while rem > 0:
            take = min(CH_R3 // 2 - 1, rem)  # -1 for the dummy pair
            n = -(-(take + 1) // 32) * 32    # pairs, 32-align (64 slots)
            chunks.append((k, pos, take, n))
            pos += take
            rem -= take
    tot_pairs = sum(c[3] for c in chunks)

    per_core = []
    for m in range(B):
        dst, src = esorted[m]
        counts = ecnts[m]
        src_slots = np.zeros((tot_sub, 128), np.int32)
        dl_slots = np.full((tot_sub, 128), -1, np.int32)
        pos = 0
        for g in range(NGRP):
            cnt = int(gcnts[m][g])
            so = goff[g] * 128
            src_slots.reshape(-1)[so:so + cnt] = src[pos:pos + cnt]
            dl_slots.reshape(-1)[so:so + cnt] = (dst[pos:pos + cnt]
                                                 - g * GT * 128)
            pos += cnt
        src_lin = src_slots.reshape(tot_sub * 128)
        # h1d row for sorted-slot s is (s%128)*NT + s//128 (partition-major)
        rows = (src_lin % 128) * NT + src_lin // 128
        srcw = np.tile(rows.reshape(-1, 16).T, (8, 1)).astype(np.int16)
        dl = dl_slots.reshape(tot_sub, 128).T.copy().astype(np.float16)

        # pool lane idx arrays ---------------------------------------------
        per_dst = r3_lists[m]
        # dsts sorted ascending; per pass k the dst list with deg > 2k
        sg = np.full(tot_pairs * 2, -1, np.int64)   # gather srcs
        dg = np.full(tot_pairs, -1, np.int64)       # scatter dsts (local)
        base = 0
        dum_loc = DUMT - R3T0 * 128
        for (k, pstart, take, n) in chunks:
            dlist = sorted(d for d, v in per_dst.items() if len(v) > 2 * k)
            sel = dlist[pstart:pstart + take]
            # dummy pair at slot 0
            sg[2 * base] = ZCOL
            sg[2 * base + 1] = ZCOL
            dg[base] = dum_loc
            for j, d in enumerate(sel):
                v = per_dst[d]
                s0 = v[2 * k]
                s1 = v[2 * k + 1] if len(v) > 2 * k + 1 else ZCOL
                sg[2 * (base + 1 + j)] = s0
                sg[2 * (base + 1 + j) + 1] = s1
                dg[base + 1 + j] = d
            base += n
        srcg = _wrap16(sg)
        dstg = _wrap16(dg)

        # sampling blocks ---------------------------------------------------
        wsc = np.zeros((npair, 128, 512), np.float32)
        pi = 0
        for c in range(NVCH):
            lo, hi = c * 512, min((c + 1) * 512, V)
            for mi in range(4):
                seen = set()
                for t in sched[mi][c]:
                    blk = wsc[pi]
                    if lo < V and t not in seen:
                        seen.add(t)
                        for (pix, w) in corners_all[m][mi]:
                            px = pix[lo:hi]
                            sel2 = (px >= t * 128) & (px < (t + 1) * 128)
                            jj = np.nonzero(sel2)[0]
                            np.add.at(blk, (px[jj] - t * 128, jj),
                                      w[lo:hi][jj])
                    pi += 1
        assert pi == npair

        vt = np.zeros((3, VP), np.float32)
        vt[:, :V] = verts[m * V:(m + 1) * V][sigmas[m]].T

        bf = ml_dtypes.bfloat16
        aux = {
            "f1": np.ascontiguousarray(feats[0][m].reshape(256, -1)).astype(bf),
            "f2": np.ascontiguousarray(feats[1][m].reshape(512, -1)).astype(bf),
            "f3": np.ascontiguousarray(feats[2][m].reshape(1024, -1)).astype(bf),
            "f4": np.ascontiguousarray(feats[3][m].reshape(2048, -1)).astype(bf),
            "bw": np.ascontiguousarray(
                np.asarray(inputs["bottleneck_w"], np.float32)).astype(bf),
            "wsc": wsc.reshape(npair * 128, 512).astype(bf),
            "srcw": np.ascontiguousarray(srcw),
            "dstloc": np.ascontiguousarray(dl),
            **({"srcg": np.ascontiguousarray(srcg),
                "dstg": np.ascontiguousarray(dstg)} if tot_pairs else {}),
            "iota": np.tile(np.arange(GT * 128, dtype=np.float16),
                            (128, 1)),
            "vertsT": vt.astype(bf),
            "encc": enc[m].reshape(2, 128).T.copy().astype(bf),  # [128, 2]
            "g0w0m": np.asarray(inputs["g0_w0"][:128], np.float32).astype(bf),
            "g0w0v": np.asarray(inputs["g0_w0"][128:131], np.float32).astype(bf),
            "g0w0e": np.ascontiguousarray(
                np.asarray(inputs["g0_w0"][131:387], np.float32)).astype(bf),
            "g0w1m": np.asarray(inputs["g0_w1"][:128], np.float32).astype(bf),
            "g0w1v": np.asarray(inputs["g0_w1"][128:131], np.float32).astype(bf),
            "g0w1e": np.ascontiguousarray(
                np.asarray(inputs["g0_w1"][131:387], np.float32)).astype(bf),
            "gw0": np.ascontiguousarray(
                np.asarray(inputs["gw0"], np.float32).transpose(1, 0, 2)
                .reshape(128, 7 * 128)).astype(bf),
            "gw1": np.ascontiguousarray(
                np.asarray(inputs["gw1"], np.float32).transpose(1, 0, 2)
                .reshape(128, 7 * 128)).astype(bf),
            "offw": np.asarray(inputs["off_w"], np.float32).astype(bf),
        }
        per_core.append(aux)

    cfg = {"sched": sched, "np_list": np_list, "npair": npair,
           "g_off": g_off.tolist(), "ntile_map": ntile_map,
           "ng_g": ng_g.tolist(), "goff": goff.tolist(),
           "spans": spans, "span_tot": span_tot, "g_res": G_RES,
           "res_slots": res_slots,
           "tot_sub": tot_sub, "chunks": chunks, "tot_pairs": tot_pairs}
    post = {"sigmas": sigmas}
    return cfg, per_core, post


def _build(cfg, shapes, dump=None, nlayers=8, repeat=1):
    nc = bacc.Bacc("TRN2", target_bir_lowering=False, debug=False, num_devices=B)
    ap = {}
    for name, arr in shapes.items():
        ap[name] = nc.dram_tensor(
            name, list(arr.shape), mybir.dt.from_np(arr.dtype),
            kind="ExternalInput").ap()
    out = nc.dram_tensor("out", [VP, 3], F32, kind="ExternalOutput").ap()
    xdump = (nc.dram_tensor("xdump", [128, VP], F32, kind="ExternalOutput").ap()
             if dump else None)
    h1d2 = [nc.dram_tensor("h1da", [VP, HID], BF16).ap(),
            nc.dram_tensor("h1db", [VP, HID], BF16).ap()]

    sched = cfg["sched"]
    g_off = cfg["g_off"]
    ntile_map = cfg["ntile_map"]
    NGT = g_off[4]
    tot_sub = cfg["tot_sub"]
    ng_g = cfg["ng_g"]
    goff = cfg["goff"]
    spans = cfg["spans"]
    span_tot = cfg["span_tot"]
    G_RES = cfg["g_res"]
    res_slots = cfg["res_slots"]
    ngmax = max(ng_g)
    ohmax = max(span_tot)
    chunks = cfg["chunks"]
    tot_pairs = cfg["tot_pairs"]

    with tile.TileContext(nc) as tc, ExitStack() as ctx:
        # ---------------- persistent pools ----------------
        pp = ctx.enter_context(tc.tile_pool(name="pers", bufs=1))
        xa = pp.tile([128, VP], BF16, tag="xa")
        xb = pp.tile([128, VP], BF16, tag="xb")
        h1c = None
        if tot_pairs:
            h1c = pp.tile([128, VP], F32, tag="h1c")
        srcw_t = pp.tile([128, tot_sub * 8], I16, tag="srcw")
        dstloc_t = pp.tile([128, tot_sub, 1], FP16, tag="dstloc")
        if tot_pairs:
            aggb = pp.tile([128, R3V, 2], BF16, tag="aggb")
            srcg_t = pp.tile([128, tot_pairs // 8], I16, tag="srcg")
            dstg_t = pp.tile([128, tot_pairs // 16], I16, tag="dstg")
        iota_t = pp.tile([128, GT, 128], FP16, tag="iota")
        ohres_t = pp.tile([128, max(res_slots, 1), 128], BF16,
                          tag="ohres")
        w0_t = pp.tile([128, 7 * 128], BF16, tag="w0")
        w1_t = pp.tile([128, 7 * 128], BF16, tag="w1")
        g0_t = pp.tile([128, 6 * 128], BF16, tag="g0")
        g0v_t = pp.tile([3, 256], BF16, tag="g0v")
        offw_t = pp.tile([128, 3], BF16, tag="offw")
        ones_t = pp.tile([1, GT * 128], BF16, tag="ones")
        erow_t = pp.tile([1, 256], BF16, tag="erow")
        encc_t = pp.tile([128, 2], BF16, tag="encc")

        nc.sync.dma_start(srcw_t[:], ap["srcw"][:])
        nc.sync.dma_start(
            dstloc_t[:], ap["dstloc"].rearrange("p (s o) -> p s o", o=1))
        if tot_pairs:
            nc.sync.dma_start(srcg_t[:], ap["srcg"][:])
            nc.sync.dma_start(dstg_t[:], ap["dstg"][:])
        nc.sync.dma_start(iota_t[:].rearrange("p o d -> p (o d)"),
                          ap["iota"][:])

        def _build_oh(g, out_t, out_base):
            """is_equal one-hots for group g's tile spans into out_t."""
            s0 = goff[g]
            offs = []
            off = out_base
            for ti in range(GT):
                sp = spans[g][ti]
                if sp is None:
                    continue
                lo, hi = sp
                nc.vector.tensor_tensor(
                    out=out_t[:, off:off + hi - lo, :],
                    in0=dstloc_t[:, s0 + lo:s0 + hi, :]
                    .to_broadcast([128, hi - lo, 128]),
                    in1=iota_t[:, ti:ti + 1, :]
                    .to_broadcast([128, hi - lo, 128]),
                    op=mybir.AluOpType.is_equal)
                offs.append((ti, lo, hi, off))
                off += hi - lo
            return offs

        res_offs = {}
        rbase = 0
        for g in range(G_RES):
            res_offs[g] = _build_oh(g, ohres_t, rbase)
            rbase += span_tot[g]
        nc.sync.dma_start(w0_t[:], ap["gw0"][:])
        nc.sync.dma_start(w1_t[:], ap["gw1"][:])
        nc.sync.dma_start(g0_t[:, 0:128], ap["g0w0m"][:])
        nc.sync.dma_start(g0_t[:, 128:256], ap["g0w1m"][:])
        nc.sync.dma_start(
            g0_t[:, 256:512].rearrange("p (c h) -> p c h", h=128),
            ap["g0w0e"].rearrange("(c p) h -> p c h", p=128))
        nc.sync.dma_start(
            g0_t[:, 512:768].rearrange("p (c h) -> p c h", h=128),
            ap["g0w1e"].rearrange("(c p) h -> p c h", p=128))
        nc.sync.dma_start(g0v_t[:, 0:128], ap["g0w0v"][:])
        nc.sync.dma_start(g0v_t[:, 128:256], ap["g0w1v"][:])
        nc.sync.dma_start(offw_t[:], ap["offw"][:])
        nc.vector.memset(ones_t[:], 1.0)
        nc.sync.dma_start(encc_t[:], ap["encc"][:])

        psA = ctx.enter_context(tc.tile_pool(name="psA", bufs=2, space="PSUM"))

        # enc rank-1 rows: e{0,1} = g0_w{0,1}[131:387].T @ enc  -> [1,128]
        for k in range(2):
            pe = psA.tile([1, 128], F32, tag="p1")
            for cchunk in range(2):
                nc.tensor.matmul(
                    out=pe[:],
                    lhsT=encc_t[:, cchunk:cchunk + 1],
                    rhs=g0_t[:, 256 + k * 256 + cchunk * 128:
                             256 + k * 256 + cchunk * 128 + 128],
                    start=(cchunk == 0), stop=(cchunk == 1))
            nc.scalar.activation(erow_t[:, k * 128:(k + 1) * 128], pe[:],
                                 AF.Copy)

        def _sampling():
            with ExitStack() as sctx:
                sp = sctx.enter_context(tc.tile_pool(name="samp", bufs=1))
                spf = sctx.enter_context(tc.tile_pool(name="sampf", bufs=3))
                spw = sctx.enter_context(tc.tile_pool(name="sampw", bufs=2))
                spp1 = sctx.enter_context(
                    tc.tile_pool(name="samppsum1", bufs=2, space="PSUM"))
                g_sb = sp.tile([128, NGT * 128], BF16, tag="gsb")
                for mi, (C, Wm) in enumerate(MAPS):
                    HW = Wm * Wm
                    ncc = C // 128
                    bw_t = spf.tile([128, 16 * 128], BF16, tag="bw")
                    nc.sync.dma_start(
                        bw_t[:, :ncc * 128].rearrange("p (c h) -> p c h",
                                                      h=128),
                        ap["bw"].rearrange("(c p) h -> p c h", p=128)
                        [:, CH_OFF[mi] // 128:CH_OFF[mi] // 128 + ncc, :])
                    fm_t = sp.tile([128, 2 * 3136], BF16, tag="fm")
                    nc.sync.dma_start(
                        fm_t[:, :ncc * HW].rearrange("p (c hw) -> p c hw",
                                                     c=ncc),
                        ap[f"f{mi+1}"].rearrange("(c p) hw -> p c hw", p=128))
                    for t in range(ntile_map[mi]):
                        p0 = t * 128
                        pcnt = min(128, HW - p0)
                        pg = psA.tile([128, 128], F32, tag="p1")
                        for cc in range(ncc):
                            nc.tensor.matmul(
                                out=pg[:pcnt, :],
                                lhsT=fm_t[:, cc * HW + p0:cc * HW + p0 + pcnt],
                                rhs=bw_t[:, cc * 128:cc * 128 + 128],
                                start=(cc == 0), stop=(cc == ncc - 1))
                        gt = g_off[mi] + t
                        nc.scalar.activation(
                            g_sb[:pcnt, gt * 128:gt * 128 + 128], pg[:pcnt, :],
                            AF.Copy)

                npc = sum(len(sched[mi][0]) for mi in range(4))
                for c in range(NVCH):
                    ps = spp1.tile([128, 512], F32, tag="ps")
                    pairs_c = []
                    for mi in range(4):
                        for t in sched[mi][c]:
                            pairs_c.append((mi, t))
                    assert len(pairs_c) == npc
                    half = (npc + 1) // 2
                    wts = []
                    for hb in range(2):
                        k0, k1 = hb * half, min((hb + 1) * half, npc)
                        wt = spw.tile([128, half, 512], BF16, tag="wsc")
                        nc.sync.dma_start(
                            wt[:, :k1 - k0, :],
                            ap["wsc"].rearrange("(k p) h -> p k h", p=128)
                            [:, c * npc + k0:c * npc + k1, :])
                        wts.append(wt)
                    for k, (mi, t) in enumerate(pairs_c):
                        HW = MAPS[mi][1] ** 2
                        pcnt = min(128, HW - t * 128)
                        gt = g_off[mi] + t
                        nc.tensor.matmul(
                            out=ps[:],
                            lhsT=g_sb[:pcnt, gt * 128:gt * 128 + 128],
                            rhs=wts[k // half][:pcnt, k % half, :],
                            start=(k == 0), stop=(k == len(pairs_c) - 1))
                    nc.scalar.activation(xa[:, c * 512:(c + 1) * 512], ps[:],
                                         AF.Relu)

        def _layers(lctx):
            lp = lctx.enter_context(tc.tile_pool(name="lay", bufs=2))
            lph = lctx.enter_context(tc.tile_pool(name="layh", bufs=2))
            lpm = lctx.enter_context(tc.tile_pool(name="laym", bufs=2))
            psx = lctx.enter_context(tc.tile_pool(name="psumx", bufs=2,
                                                  space="PSUM"))
            psc = lctx.enter_context(tc.tile_pool(name="psumc", bufs=2,
                                                  space="PSUM"))
            cur, nxt = xa, xb
            for l in range(nlayers):
                h1d = h1d2[l % 2]
                # ---- h1 column form [128, VP] f32 (pool lane source) ----
                for c in range(NVCH if tot_pairs else 0):
                    pc = psc.tile([128, 512], F32, tag="pc")
                    cs = c * 512
                    if l == 0:
                        nc.tensor.matmul(
                            out=pc[:], lhsT=g0_t[:, 128:256],
                            rhs=cur[:, cs:cs + 512], start=True, stop=False)
                        nc.tensor.matmul(
                            out=pc[:], lhsT=g0v_t[:, 128:256],
                            rhs=vertsT_t[:, cs:cs + 512],
                            start=False, stop=False)
                        nc.tensor.matmul(
                            out=pc[:], lhsT=erow_t[:, 128:256],
                            rhs=ones_t[:, 0:512], start=False, stop=True)
                    else:
                        nc.tensor.matmul(
                            out=pc[:], lhsT=w1_t[:, (l - 1) * 128:l * 128],
                            rhs=cur[:, cs:cs + 512], start=True, stop=True)
                    nc.scalar.activation(h1c[:, cs:cs + 512], pc[:], AF.Copy)
                # zero column for pool-lane padding
                zc = (nc.vector.memset(h1c[:, ZCOL:ZCOL + 1], 0.0)
                      if tot_pairs else None)

                # ---- h1 rows -> h1d (DMA lane source) ----
                h1_writes = []
                for t0 in range(0, NT, HB):
                    tb = min(HB, NT - t0)
                    hst = lph.tile([128, HB * 128], BF16, tag="hst")
                    for ti in range(tb):
                        t = t0 + ti
                        ph = psA.tile([128, 128], F32, tag="p1")
                        if l == 0:
                            nc.tensor.matmul(
                                out=ph[:], lhsT=cur[:, t * 128:(t + 1) * 128],
                                rhs=g0_t[:, 128:256], start=True, stop=False)
                            nc.tensor.matmul(
                                out=ph[:],
                                lhsT=vertsT_t[:, t * 128:(t + 1) * 128],
                                rhs=g0v_t[:, 128:256], start=False, stop=False)
                            nc.tensor.matmul(
                                out=ph[:], lhsT=ones_t[:, 0:128],
                                rhs=erow_t[:, 128:256], start=False, stop=True)
                        else:
                            nc.tensor.matmul(
                                out=ph[:], lhsT=cur[:, t * 128:(t + 1) * 128],
                                rhs=w1_t[:, (l - 1) * 128:l * 128],
                                start=True, stop=True)
                        nc.scalar.activation(hst[:, ti * 128:(ti + 1) * 128],
                                             ph[:], AF.Copy)
                    h1_writes.append(nc.sync.dma_start(
                        h1d.rearrange("(p n) c -> p n c", p=128)
                        [:, t0:t0 + tb, :],
                        hst[:, :tb * 128].rearrange("p (n c) -> p n c",
                                                    c=128)))

                # ---- DMA lane: gather groups + one-hot scatter matmuls ----
                for g in range(NGRP):
                    s0, s1 = goff[g], goff[g + 1]
                    ng = s1 - s0
                    msg = lp.tile([128, ngmax, 128], BF16, tag="msg")
                    gi = nc.gpsimd.dma_gather(
                        out_ap=msg[:, :ng, :],
                        in_ap=h1d[:],
                        idxs_ap=srcw_t[:, s0 * 8:s1 * 8],
                        num_idxs=ng * 128,
                        num_idxs_reg=ng * 128,
                        elem_size=HID,
                        single_packet=False,
                    )
                    for wi in h1_writes:
                        tile.add_dep_helper(gi.ins, wi.ins,
                                            reason="h1 RAW: gather after write")
                    if g < G_RES:
                        oh, offs = ohres_t, res_offs[g]
                    else:
                        oh = lp.tile([128, ohmax, 128], BF16, tag="oh")
                        offs = _build_oh(g, oh, 0)
                    W = GT * 128
                    px = psx.tile([128, W], F32, tag="px")
                    if l == 0:
                        nc.tensor.matmul(
                            out=px[:], lhsT=g0_t[:, 0:128],
                            rhs=cur[:, g * W:(g + 1) * W],
                            start=True, stop=False)
                        nc.tensor.matmul(
                            out=px[:], lhsT=g0v_t[:, 0:128],
                            rhs=vertsT_t[:, g * W:(g + 1) * W],
                            start=False, stop=False)
                        nc.tensor.matmul(
                            out=px[:], lhsT=erow_t[:, 0:128],
                            rhs=ones_t[:], start=False, stop=False)
                    else:
                        nc.tensor.matmul(
                            out=px[:], lhsT=w0_t[:, (l - 1) * 128:l * 128],
                            rhs=cur[:, g * W:(g + 1) * W],
                            start=True, stop=False)
                    ops = []
                    for (ti, lo, hi, off) in offs:
                        for s in range(lo, hi):
                            ops.append((ti, s, off + s - lo))
                    for oi, (ti, s, oo) in enumerate(ops):
                        nc.tensor.matmul(
                            out=px[:, ti * 128:(ti + 1) * 128],
                            lhsT=msg[:, s, :], rhs=oh[:, oo, :],
                            start=False,
                            stop=(oi == len(ops) - 1),
                            skip_group_check=True)
                    nc.scalar.activation(nxt[:, g * W:(g + 1) * W], px[:],
                                         AF.Relu)

                # ---- Pool lane: h0 init + gather/scatter passes ----
                for hh in range(R3V // 512 if tot_pairs else 0):
                    ph0 = psx.tile([128, 512], F32, tag="px")
                    c0 = R3T0 * 128 + hh * 512
                    if l == 0:
                        nc.tensor.matmul(
                            out=ph0[:], lhsT=g0_t[:, 0:128],
                            rhs=cur[:, c0:c0 + 512], start=True, stop=False)
                        nc.tensor.matmul(
                            out=ph0[:], lhsT=g0v_t[:, 0:128],
                            rhs=vertsT_t[:, c0:c0 + 512],
                            start=False, stop=False)
                        nc.tensor.matmul(
                            out=ph0[:], lhsT=erow_t[:, 0:128],
                            rhs=ones_t[:, 0:512], start=False, stop=True)
                    else:
                        nc.tensor.matmul(
                            out=ph0[:], lhsT=w0_t[:, (l - 1) * 128:l * 128],
                            rhs=cur[:, c0:c0 + 512], start=True, stop=True)
                    nc.scalar.activation(
                        aggb[:, hh * 512:(hh + 1) * 512, 0:1]
                        .rearrange("p n d -> p (n d)"), ph0[:], AF.Copy)
                if tot_pairs:
                    zm = nc.vector.memset(
                        aggb[:, :, 1:2].rearrange("p n d -> p (n d)"), 0.0)

                scs = []
                base = 0
                for (k, pstart, take, n) in chunks:
                    mf = lpm.tile([128, CH_R3], F32, tag="mf")
                    ga = nc.gpsimd.ap_gather(
                        out_ap=mf[:, :2 * n].rearrange("p (n d) -> p n d",
                                                       d=1),
                        in_ap=h1c[:].rearrange("p (n d) -> p n d", d=1),
                        idxs_ap=srcg_t[:, base // 8:(base + n) // 8],
                        channels=128, num_elems=VP, d=1, num_idxs=2 * n,
                    )
                    tile.add_dep_helper(ga.ins, zc.ins, reason="zero col")
                    ma = lpm.tile([128, CH_R3 // 2, 2], BF16, tag="ma")
                    nc.scalar.activation(
                        ma[:, :n, :].rearrange("p n d -> p (n d)"),
                        mf[:, :2 * n], AF.Copy)
                    sc = nc.gpsimd.scatter_add(
                        in_ap=aggb[:],
                        idxs_ap=dstg_t[:, base // 16:(base + n) // 16],
                        add_ap=ma[:, :n, :],
                        channels=128, num_elems=R3V, d=2, num_idxs=n,
                    )
                    tile.add_dep_helper(sc.ins, zm.ins, reason="lane1 zero")
                    scs.append(sc)
                    base += n

                # lanes sum + relu -> nxt pool-region
                if not tot_pairs:
                    cur, nxt = nxt, cur
                    continue
                lsum = lpm.tile([128, R3V], BF16, tag="lsum")
                tt = nc.vector.tensor_tensor(
                    out=lsum[:],
                    in0=aggb[:, :, 0:1].rearrange("p n d -> p (n d)"),
                    in1=aggb[:, :, 1:2].rearrange("p n d -> p (n d)"),
                    op=mybir.AluOpType.add)
                for sc in scs:
                    tile.add_dep_helper(tt.ins, sc.ins, reason="after scatter")
                nc.scalar.activation(nxt[:, R3T0 * 128:VP], lsum[:], AF.Relu)

                cur, nxt = nxt, cur
            return cur

        vpp = ctx.enter_context(tc.tile_pool(name="vt", bufs=1))
        vertsT_t = vpp.tile([3, VP], BF16, tag="vT")
        nc.sync.dma_start(vertsT_t[:], ap["vertsT"][:])

        for _rep in range(repeat):
            _sampling()
            with ExitStack() as lctx:
                cur = _layers(lctx)

        if xdump is not None:
            dp = ctx.enter_context(tc.tile_pool(name="dump", bufs=1))
            dt_ = dp.tile([128, VP], F32, tag="xd")
            nc.scalar.activation(dt_[:], cur[:], AF.Copy)
            nc.sync.dma_start(xdump[:], dt_[:])

        # ---------------- output ----------------
        OB = 12
        op_pool = ctx.enter_context(tc.tile_pool(name="outp", bufs=2))
        for t0 in range(0, NT, OB):
            tb = min(OB, NT - t0)
            ost = op_pool.tile([128, OB * 3], F32, tag="ost")
            for ti in range(tb):
                t = t0 + ti
                po = psA.tile([128, 3], F32, tag="p1")
                nc.tensor.matmul(out=po[:], lhsT=cur[:, t * 128:(t + 1) * 128],
                                 rhs=offw_t[:], start=True, stop=True)
                nc.scalar.activation(ost[:, ti * 3:(ti + 1) * 3], po[:],
                                     AF.Copy)
            nc.sync.dma_start(
                out.rearrange("(n p) c -> p n c", p=128)[:, t0:t0 + tb, :],
                ost[:, :tb * 3].rearrange("p (n c) -> p n c", c=3))

    nc.compile()
    return nc


_CACHE = {}


def kernel(**inputs) -> np.ndarray:
    cfg, per_core, post = _prep(inputs)
    key = (cfg["npair"], tuple(cfg["np_list"]), tuple(cfg["ng_g"]),
           str(cfg["spans"]), tuple(cfg["chunks"]))
    if key not in _CACHE:
        _CACHE[key] = _build(cfg, per_core[0])
    nc = _CACHE[key]
    res = run_bass_kernel_spmd(nc, per_core, list(range(B)))
    outs = np.empty((B, V, 3), np.float32)
    for m in range(B):
        rows = res.results[m]["out"][:V]
        outs[m][post["sigmas"][m]] = rows
    return outs.reshape(B * V, 3)


if __name__ == "__main__":
    pass


# revision 16
# speedup vs baseline: 1.3216x; 1.3216x over previous
"""Trainium2 Bass kernel for DeformationNetworkGraphConvolutionalFullRes.

Full (unsharded) inputs in, full output out. Data-parallel over the 4 meshes:
core m processes mesh m (cores 4-7 idle). Inside each core:

  - vert_align sampling as (S @ F) @ W == S @ (F @ W): per feature map,
    F[C,HW] @ Wslice[C,128] -> G[HW,128] (bf16 matmuls), then the sparse
    bilinear operator S applied as dense [128px, 512vert] bf16 blocks
    (built host-side) accumulated in PSUM. Vertices pre-sorted by image cell.
  - Each GraphConv layer splits message traffic across two lanes:
      * DMA lane (dst tiles 0..R3T0-1): h1 rows -> HBM (partition-major
        contiguous layout), dma_gather pulls messages in dst-sorted order,
        segmented sum via one-hot (is_equal) matmuls accumulating in PSUM
        on top of h0, ReLU writes next activations.
      * Pool lane (last K_R3 dst tiles): h1 kept in column form [128,VP]
        f32 in SBUF; gpsimd.ap_gather pulls message columns; bf16 msg pairs
        scatter-add (d=2, unique dst per call via rank-pass decomposition)
        into a 2-lane accumulator initialized with h0; lanes summed + ReLU.
"""

import ml_dtypes
import numpy as np
from contextlib import ExitStack

import concourse.bass as bass
import concourse.tile as tile
from concourse import bacc, mybir
from concourse.bass_utils import run_bass_kernel_spmd

# ---------------- problem constants (hardcoded per spec) ----------------
B = 4
V = 10242
E_PER = 30720
HID = 128
MAPS = [(256, 56), (512, 28), (1024, 14), (2048, 7)]  # (C, H==W)
CH_OFF = [0, 256, 768, 1792, 3840]

VP = 10752            # padded vertex count: 84 tiles of 128
NT = VP // 128        # 84 vertex tiles
NVCH = VP // 512      # 21 vertex chunks (sampling)
NSUB = 7              # max edge subchunks (of 128) per dst tile
K_R3 = 0              # dst tiles handled by the Pool lane (last K_R3 tiles)
R3T0 = NT - K_R3      # first pool-lane tile
R3V = K_R3 * 128      # pool-lane vertex slots
GT = 4                # dst tiles per dma-gather group
NGRP = R3T0 // GT     # gather groups (DMA lane)
SUB_G = GT * NSUB
HB = 12               # h1 row write batch (tiles)
CH_R3 = 1024          # pool-lane msg chunk (slots, = 2*pairs)
ZCOL = VP - 2         # guaranteed-zero h1col column (global sorted-slot)
DUMT = VP - 1         # dummy dst (global sorted-slot; pad region)

F32 = mybir.dt.float32
BF16 = mybir.dt.bfloat16
FP16 = mybir.dt.float16
I32 = mybir.dt.int32
I16 = mybir.dt.int16
AF = mybir.ActivationFunctionType


def _corners(grid, W):
    x = (grid[:, 0] + 1.0) * 0.5 * (W - 1)
    y = (grid[:, 1] + 1.0) * 0.5 * (W - 1)
    x0f, y0f = np.floor(x), np.floor(y)
    wx1, wy1 = (x - x0f).astype(np.float32), (y - y0f).astype(np.float32)
    wx0, wy0 = 1.0 - wx1, 1.0 - wy1
    x0 = np.clip(x0f, 0, W - 1).astype(np.int64)
    x1 = np.clip(x0f + 1, 0, W - 1).astype(np.int64)
    y0 = np.clip(y0f, 0, W - 1).astype(np.int64)
    y1 = np.clip(y0f + 1, 0, W - 1).astype(np.int64)
    return [
        (y0 * W + x0, wy0 * wx0),
        (y0 * W + x1, wy0 * wx1),
        (y1 * W + x0, wy1 * wx0),
        (y1 * W + x1, wy1 * wx1),
    ]


def _wrap16(a):
    """[n] int -> [128, n//16] int16 (wrapped in 16 partitions, replicated)."""
    return np.tile(np.asarray(a, np.int16).reshape(-1, 16).T, (8, 1))


def _prep(inputs):
    feats = [inputs["feat1"], inputs["feat2"], inputs["feat3"], inputs["feat4"]]
    av = np.asarray(inputs["aligned_verts"], np.float32)
    verts = np.asarray(inputs["verts_packed"], np.float32)
    enc = np.asarray(inputs["image_enc"], np.float32)
    edges = np.asarray(inputs["edges"], np.int64)

    for bn in ["bottleneck_b", "g0_b0", "g0_b1", "off_b"]:
        assert not np.any(np.asarray(inputs[bn])), f"{bn} nonzero: unsupported"
    assert not np.any(np.asarray(inputs["gb0"])) and not np.any(
        np.asarray(inputs["gb1"])
    ), "gb nonzero: unsupported"

    # per-mesh vertex sort (by finest-map cell) ----------------------------
    sigmas, invs, corners_all = [], [], []
    for m in range(B):
        grid = av[m, :, :2]
        cs = _corners(grid, MAPS[0][1])
        key = cs[0][0]
        sigma = np.argsort(key, kind="stable")
        inv = np.empty(V, np.int64)
        inv[sigma] = np.arange(V)
        sigmas.append(sigma)
        invs.append(inv)
        corners_all.append(
            [[(pix[sigma], w[sigma]) for (pix, w) in _corners(grid, Wm)]
             for (_, Wm) in MAPS]
        )

    # sampling schedule ----------------------------------------------------
    ntile_map = [(Wm * Wm + 127) // 128 for (_, Wm) in MAPS]
    g_off = np.cumsum([0] + ntile_map)
    sched = []
    for mi in range(4):
        per_c = []
        for c in range(NVCH):
            lo, hi = c * 512, min((c + 1) * 512, V)
            tiles = set()
            if lo < V:
                for m in range(B):
                    for (pix, _w) in corners_all[m][mi]:
                        pc = pix[lo:hi] // 128
                        tiles.update(np.unique(pc).tolist())
            per_c.append(sorted(tiles) if tiles else [0])
        np_m = max(len(t) for t in per_c)
        per_c = [t + [t[0]] * (np_m - len(t)) for t in per_c]
        sched.append(per_c)
    np_list = [len(sched[mi][0]) for mi in range(4)]
    npair = sum(np_list) * NVCH

    # graph structure ------------------------------------------------------
    esorted = []   # per mesh: (dst, src) directed, dst-sorted (r1 part only)
    ecnts = []     # per mesh: [NT] counts (r1 tiles only meaningful)
    r3_lists = []  # per mesh: {dst_local: [srcs]} for pool-lane tiles
    for m in range(B):
        e = edges[m * E_PER:(m + 1) * E_PER] - m * V
        a = invs[m][e[:, 0]]
        b = invs[m][e[:, 1]]
        dst = np.concatenate([a, b])
        src = np.concatenate([b, a])
        r3_mask = dst >= R3T0 * 128
        d1, s1 = dst[~r3_mask], src[~r3_mask]
        order = np.lexsort((s1, d1))
        esorted.append((d1[order], s1[order]))
        ecnts.append(np.bincount(d1 // 128, minlength=NT))
        d3 = dst[r3_mask] - R3T0 * 128
        s3 = src[r3_mask]
        per_dst = {}
        o3 = np.lexsort((s3, d3))
        for dd, ss in zip(d3[o3], s3[o3]):
            per_dst.setdefault(int(dd), []).append(int(ss))
        r3_lists.append(per_dst)

    # per-group slot packing (GT tiles per group, contiguous dst-sorted)
    gcnts = [ecnts[m][:R3T0].reshape(NGRP, GT).sum(axis=1) for m in range(B)]
    ng_g = np.maximum(1, -(-np.stack(gcnts).max(axis=0) // 128))  # [NGRP]
    goff = np.concatenate([[0], np.cumsum(ng_g)]).astype(int)
    tot_sub = int(goff[-1])
    # per (group, tile) subchunk spans (union over meshes)
    spans = []
    for g in range(NGRP):
        per_tile = []
        for ti in range(GT):
            lo = hi = None
            for m in range(B):
                ct = ecnts[m][:R3T0]
                start = int(ct[g * GT:g * GT + ti].sum())
                end = start + int(ct[g * GT + ti])
                if end > start:
                    l0, h0 = start // 128, -(-end // 128)
                    lo = l0 if lo is None else min(lo, l0)
                    hi = h0 if hi is None else max(hi, h0)
            per_tile.append((lo, hi) if lo is not None else None)
        spans.append(per_tile)
    span_tot = [sum(h - l for sp in spans[g] if sp for (l, h) in [sp])
                for g in range(NGRP)]
    # resident one-hot budget: groups 0..G_RES-1 prebuilt in SBUF
    RES_MAX = 240
    G_RES = 0
    acc = 0
    for g in range(NGRP):
        if acc + span_tot[g] > RES_MAX:
            break
        acc += span_tot[g]
        G_RES = g + 1
    res_slots = sum(span_tot[:G_RES])

    # pool-lane pass/chunk schedule (SPMD-uniform sizes) -------------------
    maxdeg = 0
    for m in range(B):
        for v in r3_lists[m].values():
            maxdeg = max(maxdeg, len(v))
    npass = (maxdeg + 1) // 2
    pass_pairs = []  # per pass: max over meshes of #dsts with deg > 2k
    for k in range(npass):
        mx = 0
        for m in range(B):
            mx = max(mx, sum(1 for v in r3_lists[m].values() if len(v) > 2 * k))
        pass_pairs.append(mx)
    # chunks: (pass, pair_start, npairs) with one dummy pair at slot 0 each
    chunks = []
    for k in range(npass):
        rem = pass_pairs[k]
        pos = 0
        while rem > 0:
            take = min(CH_R3 // 2 - 1, rem)  # -1 for the dummy pair
            n = -(-(take + 1) // 32) * 32    # pairs, 32-align (64 slots)
            chunks.append((k, pos, take, n))
            pos += take
            rem -= take
    tot_pairs = sum(c[3] for c in chunks)

    per_core = []
    for m in range(B):
        dst, src = esorted[m]
        counts = ecnts[m]
        src_slots = np.zeros((tot_sub, 128), np.int32)
        dl_slots = np.full((tot_sub, 128), -1, np.int32)
        pos = 0
        for g in range(NGRP):
            cnt = int(gcnts[m][g])
            so = goff[g] * 128
            src_slots.reshape(-1)[so:so + cnt] = src[pos:pos + cnt]
            dl_slots.reshape(-1)[so:so + cnt] = (dst[pos:pos + cnt]
                                                 - g * GT * 128)
            pos += cnt
        src_lin = src_slots.reshape(tot_sub * 128)
        # h1d row for sorted-slot s is (s%128)*NT + s//128 (partition-major)
        rows = (src_lin % 128) * NT + src_lin // 128
        srcw = np.tile(rows.reshape(-1, 16).T, (8, 1)).astype(np.int16)
        dl = dl_slots.reshape(tot_sub, 128).T.copy().astype(np.float16)

        # pool lane idx arrays ---------------------------------------------
        per_dst = r3_lists[m]
        # dsts sorted ascending; per pass k the dst list with deg > 2k
        sg = np.full(tot_pairs * 2, -1, np.int64)   # gather srcs
        dg = np.full(tot_pairs, -1, np.int64)       # scatter dsts (local)
        base = 0
        dum_loc = DUMT - R3T0 * 128
        for (k, pstart, take, n) in chunks:
            dlist = sorted(d for d, v in per_dst.items() if len(v) > 2 * k)
            sel = dlist[pstart:pstart + take]
            # dummy pair at slot 0
            sg[2 * base] = ZCOL
            sg[2 * base + 1] = ZCOL
            dg[base] = dum_loc
            for j, d in enumerate(sel):
                v = per_dst[d]
                s0 = v[2 * k]
                s1 = v[2 * k + 1] if len(v) > 2 * k + 1 else ZCOL
                sg[2 * (base + 1 + j)] = s0
                sg[2 * (base + 1 + j) + 1] = s1
                dg[base + 1 + j] = d
            base += n
        srcg = _wrap16(sg)
        dstg = _wrap16(dg)

        # sampling blocks ---------------------------------------------------
        wsc = np.zeros((npair, 128, 512), np.float32)
        pi = 0
        for c in range(NVCH):
            lo, hi = c * 512, min((c + 1) * 512, V)
            for mi in range(4):
                seen = set()
                for t in sched[mi][c]:
                    blk = wsc[pi]
                    if lo < V and t not in seen:
                        seen.add(t)
                        for (pix, w) in corners_all[m][mi]:
                            px = pix[lo:hi]
                            sel2 = (px >= t * 128) & (px < (t + 1) * 128)
                            jj = np.nonzero(sel2)[0]
                            np.add.at(blk, (px[jj] - t * 128, jj),
                                      w[lo:hi][jj])
                    pi += 1
        assert pi == npair

        vt = np.zeros((3, VP), np.float32)
        vt[:, :V] = verts[m * V:(m + 1) * V][sigmas[m]].T

        bf = ml_dtypes.bfloat16
        aux = {
            "f1": np.ascontiguousarray(feats[0][m].reshape(256, -1)).astype(bf),
            "f2": np.ascontiguousarray(feats[1][m].reshape(512, -1)).astype(bf),
            "f3": np.ascontiguousarray(feats[2][m].reshape(1024, -1)).astype(bf),
            "f4": np.ascontiguousarray(feats[3][m].reshape(2048, -1)).astype(bf),
            "bw": np.ascontiguousarray(
                np.asarray(inputs["bottleneck_w"], np.float32)).astype(bf),
            "wsc": wsc.reshape(npair * 128, 512).astype(bf),
            "srcw": np.ascontiguousarray(srcw),
            "dstloc": np.ascontiguousarray(dl),
            **({"srcg": np.ascontiguousarray(srcg),
                "dstg": np.ascontiguousarray(dstg)} if tot_pairs else {}),
            "iota": np.tile(np.arange(GT * 128, dtype=np.float16),
                            (128, 1)),
            "vertsT": vt.astype(bf),
            "encc": enc[m].reshape(2, 128).T.copy().astype(bf),  # [128, 2]
            "g0w0m": np.asarray(inputs["g0_w0"][:128], np.float32).astype(bf),
            "g0w0v": np.asarray(inputs["g0_w0"][128:131], np.float32).astype(bf),
            "g0w0e": np.ascontiguousarray(
                np.asarray(inputs["g0_w0"][131:387], np.float32)).astype(bf),
            "g0w1m": np.asarray(inputs["g0_w1"][:128], np.float32).astype(bf),
            "g0w1v": np.asarray(inputs["g0_w1"][128:131], np.float32).astype(bf),
            "g0w1e": np.ascontiguousarray(
                np.asarray(inputs["g0_w1"][131:387], np.float32)).astype(bf),
            "gw0": np.ascontiguousarray(
                np.asarray(inputs["gw0"], np.float32).transpose(1, 0, 2)
                .reshape(128, 7 * 128)).astype(bf),
            "gw1": np.ascontiguousarray(
                np.asarray(inputs["gw1"], np.float32).transpose(1, 0, 2)
                .reshape(128, 7 * 128)).astype(bf),
            "offw": np.asarray(inputs["off_w"], np.float32).astype(bf),
        }
        per_core.append(aux)

    cfg = {"sched": sched, "np_list": np_list, "npair": npair,
           "g_off": g_off.tolist(), "ntile_map": ntile_map,
           "ng_g": ng_g.tolist(), "goff": goff.tolist(),
           "spans": spans, "span_tot": span_tot, "g_res": G_RES,
           "res_slots": res_slots,
           "tot_sub": tot_sub, "chunks": chunks, "tot_pairs": tot_pairs}
    post = {"sigmas": sigmas}
    return cfg, per_core, post


def _build(cfg, shapes, dump=None, nlayers=8, repeat=1):
    nc = bacc.Bacc("TRN2", target_bir_lowering=False, debug=False, num_devices=B)
    ap = {}
    for name, arr in shapes.items():
        ap[name] = nc.dram_tensor(
            name, list(arr.shape), mybir.dt.from_np(arr.dtype),
            kind="ExternalInput").ap()
    out = nc.dram_tensor("out", [VP, 3], F32, kind="ExternalOutput").ap()
    xdump = (nc.dram_tensor("xdump", [128, VP], F32, kind="ExternalOutput").ap()
             if dump else None)
    h1d2 = [nc.dram_tensor("h1da", [VP, HID], BF16).ap(),
            nc.dram_tensor("h1db", [VP, HID], BF16).ap()]

    sched = cfg["sched"]
    g_off = cfg["g_off"]
    ntile_map = cfg["ntile_map"]
    NGT = g_off[4]
    tot_sub = cfg["tot_sub"]
    ng_g = cfg["ng_g"]
    goff = cfg["goff"]
    spans = cfg["spans"]
    span_tot = cfg["span_tot"]
    G_RES = cfg["g_res"]
    res_slots = cfg["res_slots"]
    ngmax = max(ng_g)
    ohmax = max(span_tot)
    chunks = cfg["chunks"]
    tot_pairs = cfg["tot_pairs"]

    with tile.TileContext(nc) as tc, ExitStack() as ctx:
        # ---------------- persistent pools ----------------
        pp = ctx.enter_context(tc.tile_pool(name="pers", bufs=1))
        xa = pp.tile([128, VP], BF16, tag="xa")
        xb = pp.tile([128, VP], BF16, tag="xb")
        h1c = None
        if tot_pairs:
            h1c = pp.tile([128, VP], F32, tag="h1c")
        srcw_t = pp.tile([128, tot_sub * 8], I16, tag="srcw")
        dstloc_t = pp.tile([128, tot_sub, 1], FP16, tag="dstloc")
        if tot_pairs:
            aggb = pp.tile([128, R3V, 2], BF16, tag="aggb")
            srcg_t = pp.tile([128, tot_pairs // 8], I16, tag="srcg")
            dstg_t = pp.tile([128, tot_pairs // 16], I16, tag="dstg")
        iota_t = pp.tile([128, GT, 128], FP16, tag="iota")
        ohres_t = pp.tile([128, max(res_slots, 1), 128], BF16,
                          tag="ohres")
        w0_t = pp.tile([128, 7 * 128], BF16, tag="w0")
        w1_t = pp.tile([128, 7 * 128], BF16, tag="w1")
        g0_t = pp.tile([128, 6 * 128], BF16, tag="g0")
        g0v_t = pp.tile([3, 256], BF16, tag="g0v")
        offw_t = pp.tile([128, 3], BF16, tag="offw")
        ones_t = pp.tile([1, GT * 128], BF16, tag="ones")
        erow_t = pp.tile([1, 256], BF16, tag="erow")
        encc_t = pp.tile([128, 2], BF16, tag="encc")

        nc.sync.dma_start(srcw_t[:], ap["srcw"][:])
        nc.sync.dma_start(
            dstloc_t[:], ap["dstloc"].rearrange("p (s o) -> p s o", o=1))
        if tot_pairs:
            nc.sync.dma_start(srcg_t[:], ap["srcg"][:])
            nc.sync.dma_start(dstg_t[:], ap["dstg"][:])
        nc.sync.dma_start(iota_t[:].rearrange("p o d -> p (o d)"),
                          ap["iota"][:])

        def _build_oh(g, out_t, out_base):
            """is_equal one-hots for group g's tile spans into out_t."""
            s0 = goff[g]
            offs = []
            off = out_base
            for ti in range(GT):
                sp = spans[g][ti]
                if sp is None:
                    continue
                lo, hi = sp
                nc.vector.tensor_tensor(
                    out=out_t[:, off:off + hi - lo, :],
                    in0=dstloc_t[:, s0 + lo:s0 + hi, :]
                    .to_broadcast([128, hi - lo, 128]),
                    in1=iota_t[:, ti:ti + 1, :]
                    .to_broadcast([128, hi - lo, 128]),
                    op=mybir.AluOpType.is_equal)
                offs.append((ti, lo, hi, off))
                off += hi - lo
            return offs

        res_offs = {}
        rbase = 0
        for g in range(G_RES):
            res_offs[g] = _build_oh(g, ohres_t, rbase)
            rbase += span_tot[g]
        nc.sync.dma_start(w0_t[:], ap["gw0"][:])
        nc.sync.dma_start(w1_t[:], ap["gw1"][:])
        nc.sync.dma_start(g0_t[:, 0:128], ap["g0w0m"][:])
        nc.sync.dma_start(g0_t[:, 128:256], ap["g0w1m"][:])
        nc.sync.dma_start(
            g0_t[:, 256:512].rearrange("p (c h) -> p c h", h=128),
            ap["g0w0e"].rearrange("(c p) h -> p c h", p=128))
        nc.sync.dma_start(
            g0_t[:, 512:768].rearrange("p (c h) -> p c h", h=128),
            ap["g0w1e"].rearrange("(c p) h -> p c h", p=128))
        nc.sync.dma_start(g0v_t[:, 0:128], ap["g0w0v"][:])
        nc.sync.dma_start(g0v_t[:, 128:256], ap["g0w1v"][:])
        nc.sync.dma_start(offw_t[:], ap["offw"][:])
        nc.vector.memset(ones_t[:], 1.0)
        nc.sync.dma_start(encc_t[:], ap["encc"][:])

        psA = ctx.enter_context(tc.tile_pool(name="psA", bufs=2, space="PSUM"))

        # enc rank-1 rows: e{0,1} = g0_w{0,1}[131:387].T @ enc  -> [1,128]
        for k in range(2):
            pe = psA.tile([1, 128], F32, tag="p1")
            for cchunk in range(2):
                nc.tensor.matmul(
                    out=pe[:],
                    lhsT=encc_t[:, cchunk:cchunk + 1],
                    rhs=g0_t[:, 256 + k * 256 + cchunk * 128:
                             256 + k * 256 + cchunk * 128 + 128],
                    start=(cchunk == 0), stop=(cchunk == 1))
            nc.scalar.activation(erow_t[:, k * 128:(k + 1) * 128], pe[:],
                                 AF.Copy)

        def _sampling():
            with ExitStack() as sctx:
                sp = sctx.enter_context(tc.tile_pool(name="samp", bufs=1))
                spf = sctx.enter_context(tc.tile_pool(name="sampf", bufs=3))
                spw = sctx.enter_context(tc.tile_pool(name="sampw", bufs=2))
                spp1 = sctx.enter_context(
                    tc.tile_pool(name="samppsum1", bufs=2, space="PSUM"))
                g_sb = sp.tile([128, NGT * 128], BF16, tag="gsb")
                for mi, (C, Wm) in enumerate(MAPS):
                    HW = Wm * Wm
                    ncc = C // 128
                    bw_t = spf.tile([128, 16 * 128], BF16, tag="bw")
                    nc.sync.dma_start(
                        bw_t[:, :ncc * 128].rearrange("p (c h) -> p c h",
                                                      h=128),
                        ap["bw"].rearrange("(c p) h -> p c h", p=128)
                        [:, CH_OFF[mi] // 128:CH_OFF[mi] // 128 + ncc, :])
                    fm_t = sp.tile([128, 2 * 3136], BF16, tag="fm")
                    nc.sync.dma_start(
                        fm_t[:, :ncc * HW].rearrange("p (c hw) -> p c hw",
                                                     c=ncc),
                        ap[f"f{mi+1}"].rearrange("(c p) hw -> p c hw", p=128))
                    for t in range(ntile_map[mi]):
                        p0 = t * 128
                        pcnt = min(128, HW - p0)
                        pg = psA.tile([128, 128], F32, tag="p1")
                        for cc in range(ncc):
                            nc.tensor.matmul(
                                out=pg[:pcnt, :],
                                lhsT=fm_t[:, cc * HW + p0:cc * HW + p0 + pcnt],
                                rhs=bw_t[:, cc * 128:cc * 128 + 128],
                                start=(cc == 0), stop=(cc == ncc - 1))
                        gt = g_off[mi] + t
                        nc.scalar.activation(
                            g_sb[:pcnt, gt * 128:gt * 128 + 128], pg[:pcnt, :],
                            AF.Copy)

                npc = sum(len(sched[mi][0]) for mi in range(4))
                for c in range(NVCH):
                    ps = spp1.tile([128, 512], F32, tag="ps")
                    pairs_c = []
                    for mi in range(4):
                        for t in sched[mi][c]:
                            pairs_c.append((mi, t))
                    assert len(pairs_c) == npc
                    half = (npc + 1) // 2
                    wts = []
                    for hb in range(2):
                        k0, k1 = hb * half, min((hb + 1) * half, npc)
                        wt = spw.tile([128, half, 512], BF16, tag="wsc")
                        nc.sync.dma_start(
                            wt[:, :k1 - k0, :],
                            ap["wsc"].rearrange("(k p) h -> p k h", p=128)
                            [:, c * npc + k0:c * npc + k1, :])
                        wts.append(wt)
                    for k, (mi, t) in enumerate(pairs_c):
                        HW = MAPS[mi][1] ** 2
                        pcnt = min(128, HW - t * 128)
                        gt = g_off[mi] + t
                        nc.tensor.matmul(
                            out=ps[:],
                            lhsT=g_sb[:pcnt, gt * 128:gt * 128 + 128],
                            rhs=wts[k // half][:pcnt, k % half, :],
                            start=(k == 0), stop=(k == len(pairs_c) - 1))
                    nc.scalar.activation(xa[:, c * 512:(c + 1) * 512], ps[:],
                                         AF.Relu)

        def _layers(lctx):
            lp = lctx.enter_context(tc.tile_pool(name="lay", bufs=3))
            lph = lctx.enter_context(tc.tile_pool(name="layh", bufs=2))
            lpm = lctx.enter_context(tc.tile_pool(name="laym", bufs=2))
            psx = lctx.enter_context(tc.tile_pool(name="psumx", bufs=4,
                                                  space="PSUM"))
            psc = lctx.enter_context(tc.tile_pool(name="psumc", bufs=2,
                                                  space="PSUM"))
            cur, nxt = xa, xb
            for l in range(nlayers):
                h1d = h1d2[l % 2]
                # ---- h1 column form [128, VP] f32 (pool lane source) ----
                for c in range(NVCH if tot_pairs else 0):
                    pc = psc.tile([128, 512], F32, tag="pc")
                    cs = c * 512
                    if l == 0:
                        nc.tensor.matmul(
                            out=pc[:], lhsT=g0_t[:, 128:256],
                            rhs=cur[:, cs:cs + 512], start=True, stop=False)
                        nc.tensor.matmul(
                            out=pc[:], lhsT=g0v_t[:, 128:256],
                            rhs=vertsT_t[:, cs:cs + 512],
                            start=False, stop=False)
                        nc.tensor.matmul(
                            out=pc[:], lhsT=erow_t[:, 128:256],
                            rhs=ones_t[:, 0:512], start=False, stop=True)
                    else:
                        nc.tensor.matmul(
                            out=pc[:], lhsT=w1_t[:, (l - 1) * 128:l * 128],
                            rhs=cur[:, cs:cs + 512], start=True, stop=True)
                    nc.scalar.activation(h1c[:, cs:cs + 512], pc[:], AF.Copy)
                # zero column for pool-lane padding
                zc = (nc.vector.memset(h1c[:, ZCOL:ZCOL + 1], 0.0)
                      if tot_pairs else None)

                # ---- h1 rows -> h1d (DMA lane source) ----
                h1_writes = []
                for t0 in range(0, NT, HB):
                    tb = min(HB, NT - t0)
                    hst = lph.tile([128, HB * 128], BF16, tag="hst")
                    for ti in range(tb):
                        t = t0 + ti
                        ph = psA.tile([128, 128], F32, tag="p1")
                        if l == 0:
                            nc.tensor.matmul(
                                out=ph[:], lhsT=cur[:, t * 128:(t + 1) * 128],
                                rhs=g0_t[:, 128:256], start=True, stop=False)
                            nc.tensor.matmul(
                                out=ph[:],
                                lhsT=vertsT_t[:, t * 128:(t + 1) * 128],
                                rhs=g0v_t[:, 128:256], start=False, stop=False)
                            nc.tensor.matmul(
                                out=ph[:], lhsT=ones_t[:, 0:128],
                                rhs=erow_t[:, 128:256], start=False, stop=True)
                        else:
                            nc.tensor.matmul(
                                out=ph[:], lhsT=cur[:, t * 128:(t + 1) * 128],
                                rhs=w1_t[:, (l - 1) * 128:l * 128],
                                start=True, stop=True)
                        nc.scalar.activation(hst[:, ti * 128:(ti + 1) * 128],
                                             ph[:], AF.Copy)
                    h1_writes.append(nc.sync.dma_start(
                        h1d.rearrange("(p n) c -> p n c", p=128)
                        [:, t0:t0 + tb, :],
                        hst[:, :tb * 128].rearrange("p (n c) -> p n c",
                                                    c=128)))

                # ---- DMA lane: gather groups + one-hot scatter matmuls ----
                for g in range(NGRP):
                    s0, s1 = goff[g], goff[g + 1]
                    ng = s1 - s0
                    msg = lp.tile([128, ngmax, 128], BF16, tag="msg")
                    gi = nc.gpsimd.dma_gather(
                        out_ap=msg[:, :ng, :],
                        in_ap=h1d[:],
                        idxs_ap=srcw_t[:, s0 * 8:s1 * 8],
                        num_idxs=ng * 128,
                        num_idxs_reg=ng * 128,
                        elem_size=HID,
                        single_packet=False,
                    )
                    for wi in h1_writes:
                        tile.add_dep_helper(gi.ins, wi.ins,
                                            reason="h1 RAW: gather after write")
                    if g < G_RES:
                        oh, offs = ohres_t, res_offs[g]
                    else:
                        oh = lp.tile([128, ohmax, 128], BF16, tag="oh")
                        offs = _build_oh(g, oh, 0)
                    W = GT * 128
                    px = psx.tile([128, W], F32, tag="px")
                    if l == 0:
                        nc.tensor.matmul(
                            out=px[:], lhsT=g0_t[:, 0:128],
                            rhs=cur[:, g * W:(g + 1) * W],
                            start=True, stop=False)
                        nc.tensor.matmul(
                            out=px[:], lhsT=g0v_t[:, 0:128],
                            rhs=vertsT_t[:, g * W:(g + 1) * W],
                            start=False, stop=False)
                        nc.tensor.matmul(
                            out=px[:], lhsT=erow_t[:, 0:128],
                            rhs=ones_t[:], start=False, stop=False)
                    else:
                        nc.tensor.matmul(
                            out=px[:], lhsT=w0_t[:, (l - 1) * 128:l * 128],
                            rhs=cur[:, g * W:(g + 1) * W],
                            start=True, stop=False)
                    ops = []
                    for (ti, lo, hi, off) in offs:
                        for s in range(lo, hi):
                            ops.append((ti, s, off + s - lo))
                    for oi, (ti, s, oo) in enumerate(ops):
                        nc.tensor.matmul(
                            out=px[:, ti * 128:(ti + 1) * 128],
                            lhsT=msg[:, s, :], rhs=oh[:, oo, :],
                            start=False,
                            stop=(oi == len(ops) - 1),
                            skip_group_check=True)
                    nc.scalar.activation(nxt[:, g * W:(g + 1) * W], px[:],
                                         AF.Relu)

                # ---- Pool lane: h0 init + gather/scatter passes ----
                for hh in range(R3V // 512 if tot_pairs else 0):
                    ph0 = psx.tile([128, 512], F32, tag="px")
                    c0 = R3T0 * 128 + hh * 512
                    if l == 0:
                        nc.tensor.matmul(
                            out=ph0[:], lhsT=g0_t[:, 0:128],
                            rhs=cur[:, c0:c0 + 512], start=True, stop=False)
                        nc.tensor.matmul(
                            out=ph0[:], lhsT=g0v_t[:, 0:128],
                            rhs=vertsT_t[:, c0:c0 + 512],
                            start=False, stop=False)
                        nc.tensor.matmul(
                            out=ph0[:], lhsT=erow_t[:, 0:128],
                            rhs=ones_t[:, 0:512], start=False, stop=True)
                    else:
                        nc.tensor.matmul(
                            out=ph0[:], lhsT=w0_t[:, (l - 1) * 128:l * 128],
                            rhs=cur[:, c0:c0 + 512], start=True, stop=True)
                    nc.scalar.activation(
                        aggb[:, hh * 512:(hh + 1) * 512, 0:1]
                        .rearrange("p n d -> p (n d)"), ph0[:], AF.Copy)
                if tot_pairs:
                    zm = nc.vector.memset(
                        aggb[:, :, 1:2].rearrange("p n d -> p (n d)"), 0.0)

                scs = []
                base = 0
                for (k, pstart, take, n) in chunks:
                    mf = lpm.tile([128, CH_R3], F32, tag="mf")
                    ga = nc.gpsimd.ap_gather(
                        out_ap=mf[:, :2 * n].rearrange("p (n d) -> p n d",
                                                       d=1),
                        in_ap=h1c[:].rearrange("p (n d) -> p n d", d=1),
                        idxs_ap=srcg_t[:, base // 8:(base + n) // 8],
                        channels=128, num_elems=VP, d=1, num_idxs=2 * n,
                    )
                    tile.add_dep_helper(ga.ins, zc.ins, reason="zero col")
                    ma = lpm.tile([128, CH_R3 // 2, 2], BF16, tag="ma")
                    nc.scalar.activation(
                        ma[:, :n, :].rearrange("p n d -> p (n d)"),
                        mf[:, :2 * n], AF.Copy)
                    sc = nc.gpsimd.scatter_add(
                        in_ap=aggb[:],
                        idxs_ap=dstg_t[:, base // 16:(base + n) // 16],
                        add_ap=ma[:, :n, :],
                        channels=128, num_elems=R3V, d=2, num_idxs=n,
                    )
                    tile.add_dep_helper(sc.ins, zm.ins, reason="lane1 zero")
                    scs.append(sc)
                    base += n

                # lanes sum + relu -> nxt pool-region
                if not tot_pairs:
                    cur, nxt = nxt, cur
                    continue
                lsum = lpm.tile([128, R3V], BF16, tag="lsum")
                tt = nc.vector.tensor_tensor(
                    out=lsum[:],
                    in0=aggb[:, :, 0:1].rearrange("p n d -> p (n d)"),
                    in1=aggb[:, :, 1:2].rearrange("p n d -> p (n d)"),
                    op=mybir.AluOpType.add)
                for sc in scs:
                    tile.add_dep_helper(tt.ins, sc.ins, reason="after scatter")
                nc.scalar.activation(nxt[:, R3T0 * 128:VP], lsum[:], AF.Relu)

                cur, nxt = nxt, cur
            return cur

        vpp = ctx.enter_context(tc.tile_pool(name="vt", bufs=1))
        vertsT_t = vpp.tile([3, VP], BF16, tag="vT")
        nc.sync.dma_start(vertsT_t[:], ap["vertsT"][:])

        for _rep in range(repeat):
            _sampling()
            with ExitStack() as lctx:
                cur = _layers(lctx)

        if xdump is not None:
            dp = ctx.enter_context(tc.tile_pool(name="dump", bufs=1))
            dt_ = dp.tile([128, VP], F32, tag="xd")
            nc.scalar.activation(dt_[:], cur[:], AF.Copy)
            nc.sync.dma_start(xdump[:], dt_[:])

        # ---------------- output ----------------
        OB = 12
        op_pool = ctx.enter_context(tc.tile_pool(name="outp", bufs=2))
        for t0 in range(0, NT, OB):
            tb = min(OB, NT - t0)
            ost = op_pool.tile([128, OB * 3], F32, tag="ost")
            for ti in range(tb):
                t = t0 + ti
                po = psA.tile([128, 3], F32, tag="p1")
                nc.tensor.matmul(out=po[:], lhsT=cur[:, t * 128:(t + 1) * 128],
                                 rhs=offw_t[:], start=True, stop=True)
                nc.scalar.activation(ost[:, ti * 3:(ti + 1) * 3], po[:],
                                     AF.Copy)
            nc.sync.dma_start(
                out.rearrange("(n p) c -> p n c", p=128)[:, t0:t0 + tb, :],
                ost[:, :tb * 3].rearrange("p (n c) -> p n c", c=3))

    nc.compile()
    return nc


_CACHE = {}


def kernel(**inputs) -> np.ndarray:
    cfg, per_core, post = _prep(inputs)
    key = (cfg["npair"], tuple(cfg["np_list"]), tuple(cfg["ng_g"]),
           str(cfg["spans"]), tuple(cfg["chunks"]))
    if key not in _CACHE:
        _CACHE[key] = _build(cfg, per_core[0])
    nc = _CACHE[key]
    res = run_bass_kernel_spmd(nc, per_core, list(range(B)))
    outs = np.empty((B, V, 3), np.float32)
    for m in range(B):
        rows = res.results[m]["out"][:V]
        outs[m][post["sigmas"][m]] = rows
    return outs.reshape(B * V, 3)


if __name__ == "__main__":
    pass
